# revision 19
# baseline (speedup 1.0000x reference)
"""Trainium2 Bass kernel for ContextAM (sigmoid spatial attention + CBAM gate).

Algorithm: polynomial kernel-feature expansion of the sigmoid attention.

  E = Q^T K has contraction dim 8 and small magnitude (|E| < 5), so
  sigmoid(E) ~= c0 + c1 E + c3 E^3 + c5 E^5 to ~1e-3.  Writing
  G = wq_hat^T wk_hat (65x65, rank 8, biases folded via a ones row) and
  SVD G = U S V^T, define balanced factors Qt = S^1/2 U^T x_hat,
  Kt = S^1/2 V^T x_hat so E = Qt^T Kt exactly.  Then

    att[n,m] = sum_r w_r * Qt^alpha(r)[n] * Kt^alpha(r)[m]

  over all monomials alpha of degree {0,1,3,5} in 8 variables
  (1 + 8 + 120 + 792 = 921 features, padded to 1024 = 8 chunks of 128),
  w_r = c_|alpha| * multinomial(alpha).  The attention output becomes

    out = V @ att^T = (W * w)^T @ Phi(Q),  W[c,r] = sum_m V[c,m] Psi_r(K)[m]

  i.e. two dense rank-1024 matmuls; the O(N^2) sigmoid disappears.

Sharding: core = 2b + h (batch b, n/m-half h).  Each core builds K-features
for its m-half, accumulating a partial W, pair-AllReduces W (overlapped with
Q-feature construction + PE transposes of Phi), then computes its out-half.
CBAM stats are pair-AllGathered as in the reference decomposition.

Feature tiles are [128 rows, 1390 cols] bf16: kept region [0:1024) =
[const | L1 | L3 | L5 | zero pad], scratch [1024:1390) = [L2 | L4].
Level k is built from level k-1 as 8 suffix-blocks (leading variable j).
DVE/GPSIMD instruction overhead (~200ns) dominates small ops, so blocks are
batched 18 tiles per instruction (tensor_tensor, stride-0 broadcast
multiplier); only the largest block runs as per-tile tensor_scalar (4x mode).
"""

import math

import numpy as np

import concourse.bacc as bacc
import concourse.mybir as mybir
import concourse.tile as tile
from concourse import masks
from concourse.bass_utils import run_bass_kernel_spmd

F32 = mybir.dt.float32
BF16 = mybir.dt.bfloat16

B, C, H, W = 4, 64, 96, 96
N = H * W            # 9216
NH = N // 2          # 4608 columns per core
NT = NH // 128       # 36 tiles of 128 rows per half
NSPAN = 512
NSP = NH // NSPAN    # 9 spans
NCORES = 8
N_CORES = NCORES

# Fitted sigmoid polynomial (degrees 0,1,3,5), valid on |E| <= ~5.
C0, C1, C3, C5 = 0.49998020, 0.24950423, -0.019078693, 0.00097622674


def _enum_levels():
    levels = [[()], [(j,) for j in range(8)]]
    for k in range(2, 6):
        prev = levels[-1]
        cur = []
        for j in range(8):
            o = next((i for i, t in enumerate(prev) if t[0] >= j), len(prev))
            cur.extend((j,) + t for t in prev[o:])
        levels.append(cur)
    return levels


LEV = _enum_levels()
OFF = {0: 0, 1: 1, 3: 9, 5: 129}        # kept region offsets
OFF_PAD = 921
RKEPT = 1024
OFF_SCR = {2: 1024, 4: 1060}            # scratch offsets
FW = 1390
NCHUNK = 8


def _blocks():
    """(level, j, dst_off, src_off, cnt) in construction order."""
    base = {1: OFF[1], 2: OFF_SCR[2], 3: OFF[3], 4: OFF_SCR[4], 5: OFF[5]}
    out = []
    for k in range(2, 6):
        prev = LEV[k - 1]
        dst = base[k]
        for j in range(8):
            o = next((i for i, t in enumerate(prev) if t[0] >= j), len(prev))
            cnt = len(prev) - o
            if cnt <= 0:
                continue
            out.append((k, j, dst, base[k - 1] + o, cnt))
            dst += cnt
    return out


BLOCKS = _blocks()
TS_MIN = 300         # blocks >= TS_MIN: per-tile tensor_scalar / ACT scale

GRP = 18             # feature-group size (tiles per batched instruction)
NGRP = NT // GRP     # 2 groups per side
DSZ = 12             # of each group: tiles [0:DSZ) on DVE, rest on GPSIMD


def _multinom(t):
    k = len(t)
    c = math.factorial(k)
    for j in set(t):
        c //= math.factorial(t.count(j))
    return c


def _weights():
    w = np.zeros(RKEPT, np.float32)
    w[0] = C0
    cmap = {1: C1, 3: C3, 5: C5}
    for k in (1, 3, 5):
        for i, t in enumerate(LEV[k]):
            w[OFF[k] + i] = cmap[k] * _multinom(t)
    return w


def build_nc():
    nc = bacc.Bacc("TRN2", target_bir_lowering=False, debug=False,
                   enable_asserts=True, num_devices=NCORES)

    xh = nc.dram_tensor("xh", [C + 1, NH], BF16, kind="ExternalInput").ap()
    wcat = nc.dram_tensor("wcat", [C + 1, 82], BF16, kind="ExternalInput").ap()
    wr = nc.dram_tensor("wr", [128, NCHUNK], F32, kind="ExternalInput").ap()
    w1T = nc.dram_tensor("w1T", [C, 4], F32, kind="ExternalInput").ap()
    w2T = nc.dram_tensor("w2T", [4, C], F32, kind="ExternalInput").ap()

    y = nc.dram_tensor("y", [C, NH], F32, kind="ExternalOutput").ap()

    ccwa_in = nc.dram_tensor("ccwa_in", [C, RKEPT], BF16).ap()
    ccwa_out = nc.dram_tensor("ccwa_out", [C, RKEPT], BF16).ap()
    ccwb_in = nc.dram_tensor("ccwb_in", [C, RKEPT], BF16).ap()
    ccwb_out = nc.dram_tensor("ccwb_out", [C, RKEPT], BF16).ap()
    cc_in = nc.dram_tensor("cc_in", [1, 2 * C], F32).ap()
    cc_out = nc.dram_tensor("cc_out", [2, 2 * C], F32).ap()

    PAIRS = [[0, 1], [2, 3], [4, 5], [6, 7]]

    with tile.TileContext(nc) as tc:
        with (
            tc.tile_pool(name="const", bufs=1) as cpool,
            tc.tile_pool(name="feat", bufs=1) as fpool,
            tc.tile_pool(name="pp", bufs=2, space="PSUM") as ppool,
            tc.tile_pool(name="pw", bufs=1, space="PSUM") as wpool,
            tc.tile_pool(name="pt", bufs=2, space="PSUM") as tpool,
            tc.tile_pool(name="po", bufs=2, space="PSUM") as opool,
        ):
            # ---- resident SBUF ------------------------------------------
            X = cpool.tile([C + 1, NH], BF16)
            wcat_s = cpool.tile([C + 1, 82], BF16)
            wr_s = cpool.tile([128, NCHUNK], F32)
            w1_s = cpool.tile([C, 4], F32)
            w2_s = cpool.tile([4, C], F32)
            QKb = cpool.tile([128, NT * 18], BF16)   # [one Q0..7 one K0..7]
            QKs = cpool.tile([128, NT * 18], F32)    # f32 copy (ts scalars)
            VT = cpool.tile([128, NT * C], BF16)     # V^T tiles
            PHT = cpool.tile([128, NCHUNK * NH], BF16)  # Phi^T, chunk-major
            Wsb_a = cpool.tile([C, RKEPT], BF16)
            Wsb_b = cpool.tile([C, RKEPT], BF16)
            Wrd_a = cpool.tile([C, RKEPT], BF16)
            Wrd = cpool.tile([C, RKEPT], BF16)
            Wt = cpool.tile([128, NCHUNK * C], BF16)
            OUT = cpool.tile([C, NH], F32)
            idb = cpool.tile([128, 128], BF16)
            sums = cpool.tile([C, NSP], F32)
            maxs = cpool.tile([C, NSP], F32)

            nc.sync.dma_start(X[:, 0:NH // 2], xh[:, 0:NH // 2])
            nc.scalar.dma_start(X[:, NH // 2:], xh[:, NH // 2:])
            nc.sync.dma_start(wcat_s[:], wcat[:])
            nc.sync.dma_start(wr_s[:], wr[:])
            nc.sync.dma_start(w1_s[:], w1T[:])
            nc.sync.dma_start(w2_s[:], w2T[:])
            masks.make_identity(nc, idb[:])

            # ---- P0: projections [one|Q|one|K|V] ------------------------
            for g in range(9):
                pp = ppool.tile([128, 4 * 128], F32, tag="pp")
                for i in range(4):
                    t = 4 * g + i
                    nc.tensor.matmul(pp[:, i * 128:i * 128 + 82],
                                     X[:, t * 128:(t + 1) * 128], wcat_s[:],
                                     start=True, stop=True)
                src18 = pp[:].rearrange("p (i w) -> p i w", w=128)[:, :, 0:18]
                srcV = pp[:].rearrange("p (i w) -> p i w", w=128)[:, :, 18:82]
                qb = QKb[:].rearrange("p (t w) -> p t w", w=18)[:, 4 * g:4 * g + 4, :]
                qs = QKs[:].rearrange("p (t w) -> p t w", w=18)[:, 4 * g:4 * g + 4, :]
                vt = VT[:].rearrange("p (t w) -> p t w", w=C)[:, 4 * g:4 * g + 4, :]
                nc.scalar.copy(qb, src18)
                nc.scalar.copy(qs, src18)
                nc.scalar.copy(vt, srcV)

            # ---- feature construction -----------------------------------
            def build_features(fg, g0, qoff, first):
                """Features for tiles [g0, g0+GRP) into fg.
                qoff: 0 for Q side, 9 for K side.  Tiles [0:DSZ) of the
                group are built by DVE, the rest by GPSIMD (+ACT for the
                largest block) as independent dependency chains."""
                f3 = fg[:].rearrange("p (t w) -> p t w", w=FW)
                qb3 = QKb[:].rearrange("p (t w) -> p t w", w=18)
                if first:
                    nc.vector.memset(f3[:, :, OFF_PAD:RKEPT], 0.0)
                nc.vector.tensor_copy(f3[:, 0:DSZ, 0:9],
                                      qb3[:, g0:g0 + DSZ, qoff:qoff + 9])
                nc.gpsimd.tensor_copy(f3[:, DSZ:GRP, 0:9],
                                      qb3[:, g0 + DSZ:g0 + GRP, qoff:qoff + 9])
                for (k, j, dst, src, cnt) in BLOCKS:
                    if cnt >= TS_MIN:
                        for i in range(GRP):
                            t = g0 + i
                            sc = QKs[:, t * 18 + qoff + 1 + j:
                                     t * 18 + qoff + 2 + j]
                            if i < DSZ:
                                nc.vector.tensor_scalar_mul(
                                    fg[:, i * FW + dst:i * FW + dst + cnt],
                                    fg[:, i * FW + src:i * FW + src + cnt], sc)
                            else:
                                nc.scalar.activation(
                                    fg[:, i * FW + dst:i * FW + dst + cnt],
                                    fg[:, i * FW + src:i * FW + src + cnt],
                                    mybir.ActivationFunctionType.Copy,
                                    scale=sc)
                    else:
                        m3 = qb3[:, g0:g0 + GRP, qoff + 1 + j:qoff + 2 + j] \
                            .broadcast_to((128, GRP, cnt))
                        nc.vector.tensor_mul(f3[:, 0:DSZ, dst:dst + cnt],
                                             f3[:, 0:DSZ, src:src + cnt],
                                             m3[:, 0:DSZ, :])
                        nc.gpsimd.tensor_mul(f3[:, DSZ:GRP, dst:dst + cnt],
                                             f3[:, DSZ:GRP, src:src + cnt],
                                             m3[:, DSZ:GRP, :])

            # ---- P1: K-features + partial W; per-half bf16 AllReduce ----
            w0 = wpool.tile([C, 512], F32, tag="w0")
            w1p = wpool.tile([C, 512], F32, tag="w1")
            for g in range(NGRP):
                fg = fpool.tile([128, GRP * FW], BF16, tag="feat")
                build_features(fg, g * GRP, 9, first=(g == 0))
                for i in range(GRP):
                    t = g * GRP + i
                    st = (i == 0)
                    sp = (i == GRP - 1)
                    nc.tensor.matmul(w0[:], VT[:, t * C:(t + 1) * C],
                                     fg[:, i * FW:i * FW + 512],
                                     start=st, stop=sp)
                    nc.tensor.matmul(w1p[:], VT[:, t * C:(t + 1) * C],
                                     fg[:, i * FW + 512:i * FW + 1024],
                                     start=st, stop=sp)
                wsb = Wsb_a if g == 0 else Wsb_b
                cin = ccwa_in if g == 0 else ccwb_in
                cout = ccwa_out if g == 0 else ccwb_out
                nc.scalar.copy(wsb[:, 0:512], w0[:])
                nc.scalar.copy(wsb[:, 512:1024], w1p[:])
                nc.sync.dma_start(cin[:], wsb[:])
                nc.gpsimd.collective_compute(
                    "AllReduce", mybir.AluOpType.add,
                    ins=[cin.opt()], outs=[cout.opt()],
                    replica_groups=PAIRS)

            ph3 = PHT[:].rearrange("p (c n) -> p c n", n=NH)

            def wt_finalize():
                nc.sync.dma_start(Wrd_a[:], ccwa_out[:])
                nc.scalar.dma_start(Wrd[:], ccwb_out[:])
                nc.vector.tensor_add(Wrd[:], Wrd[:], Wrd_a[:])
                for c in range(NCHUNK):
                    pw = tpool.tile([128, C], BF16, tag="pt")
                    nc.tensor.transpose(pw[:], Wrd[:, c * 128:(c + 1) * 128],
                                        idb[0:C, 0:C])
                    nc.scalar.activation(Wt[:, c * C:(c + 1) * C], pw[:],
                                         mybir.ActivationFunctionType.Copy,
                                         scale=wr_s[:, c:c + 1])

            def out_spans(s0, two):
                po = opool.tile([128, NSPAN], F32, tag="po")
                for c in range(NCHUNK):
                    st = (c == 0)
                    sp = (c == NCHUNK - 1)
                    nc.tensor.matmul(po[0:C, :], Wt[:, c * C:(c + 1) * C],
                                     ph3[:, c, s0 * NSPAN:(s0 + 1) * NSPAN],
                                     start=st, stop=sp, tile_position=(0, 0))
                    if two:
                        nc.tensor.matmul(po[C:128, :], Wt[:, c * C:(c + 1) * C],
                                         ph3[:, c, (s0 + 1) * NSPAN:(s0 + 2) * NSPAN],
                                         start=st, stop=sp, tile_position=(0, 64))
                views = [(s0, po[0:C, :])]
                if two:
                    views.append((s0 + 1, po[C:128, :]))
                for sp_i, pv in views:
                    eng = nc.vector
                    eng.scalar_tensor_tensor(
                        OUT[:, sp_i * NSPAN:(sp_i + 1) * NSPAN], pv, 1.0,
                        X[0:C, sp_i * NSPAN:(sp_i + 1) * NSPAN],
                        op0=mybir.AluOpType.mult, op1=mybir.AluOpType.add,
                        accum_out=sums[:, sp_i:sp_i + 1])
                    nc.vector.reduce_max(
                        maxs[:, sp_i:sp_i + 1],
                        OUT[:, sp_i * NSPAN:(sp_i + 1) * NSPAN],
                        axis=mybir.AxisListType.X)

            # ---- P3: Q-features -> Phi^T; out matmuls interleaved -------
            for g in range(NGRP):
                fg = fpool.tile([128, GRP * FW], BF16, tag="feat")
                build_features(fg, g * GRP, 0, first=False)
                for i in range(GRP):
                    t = g * GRP + i
                    pt = tpool.tile([128, NCHUNK * 128], BF16, tag="pt")
                    for c in range(NCHUNK):
                        nc.tensor.transpose(
                            pt[:, c * 128:(c + 1) * 128],
                            fg[:, i * FW + c * 128:i * FW + (c + 1) * 128],
                            idb[:])
                    dst = ph3[:, :, t * 128:(t + 1) * 128]
                    src = pt[:].rearrange("p (c n) -> p c n", n=128)
                    nc.scalar.copy(dst, src)
                if g == 0:
                    wt_finalize()          # after ccs; overlaps group 1
            for s0 in range(0, NSP, 2):
                out_spans(s0, s0 + 1 < NSP)

            # ---- P5: stats exchange, CBAM gate, final scale -------------
            st = cpool.tile([C, 2], F32)
            nc.vector.reduce_sum(st[:, 0:1], sums[:], axis=mybir.AxisListType.X)
            nc.vector.reduce_max(st[:, 1:2], maxs[:], axis=mybir.AxisListType.X)
            nc.sync.dma_start(cc_in[0:1, 0:C], st[:, 0:1])
            nc.sync.dma_start(cc_in[0:1, C:2 * C], st[:, 1:2])
            nc.gpsimd.collective_compute(
                "AllGather", mybir.AluOpType.bypass,
                ins=[cc_in.opt()], outs=[cc_out.opt()],
                replica_groups=PAIRS)

            sums2 = cpool.tile([C, 2], F32)
            maxs2 = cpool.tile([C, 2], F32)
            nc.sync.dma_start(sums2[:, 0:1], cc_out[0:1, 0:C])
            nc.sync.dma_start(sums2[:, 1:2], cc_out[1:2, 0:C])
            nc.sync.dma_start(maxs2[:, 0:1], cc_out[0:1, C:2 * C])
            nc.sync.dma_start(maxs2[:, 1:2], cc_out[1:2, C:2 * C])

            avgmx = cpool.tile([C, 2], F32)
            nc.vector.reduce_sum(avgmx[:, 0:1], sums2[:], axis=mybir.AxisListType.X)
            nc.vector.tensor_scalar_mul(avgmx[:, 0:1], avgmx[:, 0:1], 1.0 / N)
            nc.vector.reduce_max(avgmx[:, 1:2], maxs2[:], axis=mybir.AxisListType.X)

            phh = opool.tile([4, 2], F32, tag="po")
            nc.tensor.matmul(phh[:], w1_s[:], avgmx[:], start=True, stop=True)
            hrelu = cpool.tile([4, 2], F32)
            nc.vector.tensor_scalar_max(hrelu[:], phh[:], 0.0)
            ps = opool.tile([C, 2], F32, tag="po")
            nc.tensor.matmul(ps[:], w2_s[:], hrelu[:], start=True, stop=True)
            ssum = cpool.tile([C, 1], F32)
            nc.vector.reduce_sum(ssum[:], ps[:], axis=mybir.AxisListType.X)
            scale = cpool.tile([C, 1], F32)
            nc.scalar.activation(scale[:], ssum[:],
                                 mybir.ActivationFunctionType.Sigmoid)

            QTR = NH // 4
            for q in range(4):
                sl = OUT[:, q * QTR:(q + 1) * QTR]
                meng = nc.vector if q % 2 == 0 else nc.gpsimd
                meng.tensor_scalar_mul(sl, sl, scale[:])
                eng = nc.sync if q % 2 == 0 else nc.scalar
                eng.dma_start(y[:, q * QTR:(q + 1) * QTR], sl)

    nc.compile()
    return nc


_NC_CACHE = None


def _get_nc():
    global _NC_CACHE
    if _NC_CACHE is None:
        _NC_CACHE = build_nc()
    return _NC_CACHE


def build_in_maps(inputs):
    import ml_dtypes
    bf16 = ml_dtypes.bfloat16

    x = np.ascontiguousarray(np.asarray(inputs["x"], np.float32))
    wq = np.asarray(inputs["wq"], np.float32)
    bq = np.asarray(inputs["bq"], np.float32)
    wk = np.asarray(inputs["wk"], np.float32)
    bk = np.asarray(inputs["bk"], np.float32)
    wv = np.asarray(inputs["wv"], np.float32)
    bv = np.asarray(inputs["bv"], np.float32)
    ca_w1 = np.asarray(inputs["ca_w1"], np.float32)
    ca_w2 = np.asarray(inputs["ca_w2"], np.float32)

    wq_h = np.concatenate([wq, bq[:, None]], axis=1)     # [8, 65]
    wk_h = np.concatenate([wk, bk[:, None]], axis=1)
    G = wq_h.T @ wk_h
    U, S, Vt = np.linalg.svd(G)
    wqt = np.sqrt(S[:8])[:, None] * U[:, :8].T           # [8, 65]
    wkt = np.sqrt(S[:8])[:, None] * Vt[:8, :]

    e1 = np.zeros((C + 1, 1), np.float32)
    e1[C, 0] = 1.0
    wv_h = np.concatenate([wv, bv[None, :].T], axis=1).T  # [65, 64]
    wcat = np.concatenate([e1, wqt.T, e1, wkt.T, wv_h], axis=1)  # [65, 82]
    wcat_b = np.ascontiguousarray(wcat.astype(bf16))

    wr = np.ascontiguousarray(_weights().reshape(NCHUNK, 128).T)
    w1T = np.ascontiguousarray(ca_w1.T)
    w2T = np.ascontiguousarray(ca_w2.T)

    xf = x.reshape(B, C, N)
    ones = np.ones((1, N), np.float32)
    in_maps = []
    for core in range(NCORES):
        b, h = core // 2, core % 2
        xb1 = np.concatenate([xf[b], ones], axis=0)[:, h * NH:(h + 1) * NH]
        in_maps.append({
            "xh": np.ascontiguousarray(xb1.astype(bf16)),
            "wcat": wcat_b, "wr": wr, "w1T": w1T, "w2T": w2T,
        })
    return in_maps


def assemble_output(results):
    out = np.empty((B, C, N), np.float32)
    for core in range(NCORES):
        b, h = core // 2, core % 2
        out[b][:, h * NH:(h + 1) * NH] = results[core]["y"]
    return out.reshape(B, C, H, W)


def kernel(**inputs):
    nc = _get_nc()
    res = run_bass_kernel_spmd(nc, build_in_maps(inputs), list(range(NCORES)))
    return assemble_output(res.results)
